# revision 24
# baseline (speedup 1.0000x reference)
"""Distributed Trainium2 kernel for the 3-branch masked attention problem.

Sharding: 8 cores; core c handles batch b = c//2 and heads h0 = 4*(c%2) .. +4
(data + head parallel).  Each core computes QKV for its heads, the three
branch softmaxes and AV locally, then a pair-wise AllGather of the [256, 1024]
attention output (transposed) per token half lets both cores of a batch apply
the output projection; the host reads even cores' outputs.

Pipeline design (v2): the attention inner loop is ACT(exp)-bound at
~2 us / j-step (2 exps of [128,1024]).  Everything else is organized to fit
under that: row-group-concurrent dots, one 4-head mask multiply per j-step on
DVE, and a one-block deferred epilogue (PE transposes) flushed at the top of
the next block so the o_ps PSUM rotation never stalls the exp stream.
"""

import numpy as np
import ml_dtypes

BF16 = ml_dtypes.bfloat16

H = 8
DA, DP, DK = 2048, 1024, 1024
B, N = 4, 2048
DOUT = 512
H_LOC = 4           # heads per core
DA_H, DP_H, DK_H = DA // H, DP // H, DK // H      # 256, 128, 128
da, dp, dk = DA_H // H, DP_H // H, DK_H // H      # 32, 16, 16
DV = da + dp + dk                                 # 64
NCORES = 8

IB = 512            # query block (moving dim of dots / AV)
JB = 128            # key chunk (contract chunk of AV, M of dots)
NI = N // IB        # 4
NJ = N // JB        # 16

_CACHE = {}


def _build():
    import concourse.bass as bass
    import concourse.mybir as mybir
    import concourse.tile as tile
    from concourse import bacc
    from concourse.masks import make_identity
    from concourse.tile import add_dep_helper

    f32 = mybir.dt.float32
    bf16 = mybir.dt.bfloat16
    Exp = mybir.ActivationFunctionType.Exp
    mult = mybir.AluOpType.mult
    add = mybir.AluOpType.add

    nc = bacc.Bacc("TRN2", target_bir_lowering=False, debug=False,
                   enable_asserts=False, num_devices=NCORES)

    xT = nc.dram_tensor("xT", [DA + DP + DK, N], bf16, kind="ExternalInput")
    maskT = nc.dram_tensor("maskT", [N, N], bf16, kind="ExternalInput")
    waT = nc.dram_tensor("waT", [DA, 384], bf16, kind="ExternalInput")
    wpkT = nc.dram_tensor("wpkT", [DP + DK, 640], bf16, kind="ExternalInput")
    woutT = nc.dram_tensor("woutT", [DOUT, DOUT], bf16, kind="ExternalInput")
    bout = nc.dram_tensor("bout", [DOUT, 1], f32, kind="ExternalInput")
    out = nc.dram_tensor("out", [DOUT, N], bf16, kind="ExternalOutput")

    with tile.TileContext(nc) as tc:
        with (
            tc.tile_pool(name="const", bufs=1) as cpool,
            tc.tile_pool(name="dram", bufs=1, space="DRAM") as dpool,
        ):
            # ---- constants ----
            ident_bf = cpool.tile([128, 128], bf16)
            make_identity(nc, ident_bf)

            bias_sb = cpool.tile([128, 4], f32)
            for t in range(4):
                nc.sync.dma_start(bias_sb[:, t:t + 1], bout[128 * t:128 * (t + 1), :])

            wa_sb = [cpool.tile([128, 384], bf16, name=f"wa{f}") for f in range(16)]
            for f in range(16):
                nc.sync.dma_start(wa_sb[f][:], waT[128 * f:128 * (f + 1), :])
            wpk_sb = [cpool.tile([128, 640], bf16, name=f"wpk{f}") for f in range(16)]
            for f in range(16):
                nc.sync.dma_start(wpk_sb[f][:], wpkT[128 * f:128 * (f + 1), :])
            wo_sb = [cpool.tile([128, DOUT], bf16, name=f"wo{f}") for f in range(4)]
            for f in range(4):
                nc.sync.dma_start(wo_sb[f][:], woutT[128 * f:128 * (f + 1), :])

            # ---- persistent activations ----
            # qT/kT per branch: [128, N]; heads live at 32-aligned partition
            # bases (p/k branches use rows 32h..32h+16)
            qTa = cpool.tile([128, N], bf16)
            kTa = cpool.tile([128, N], bf16)
            qTp = cpool.tile([128, N], bf16)
            kTp = cpool.tile([128, N], bf16)
            qTk = cpool.tile([128, N], bf16)
            kTk = cpool.tile([128, N], bf16)
            # V_aug packed per head PAIR: 16 chunks of 136 cols:
            # [vA(64) | onesA | vB(64) | onesB | pad(6)] (+64 tail pad).
            # Head h=2c+q reads the [128, 128] window at 136*j + 65*q:
            # out rows 0:64 = o, row 64 = denominator, rows 65:128 junk.
            CW = 136
            vaug = [cpool.tile([128, CW * NJ + 64], bf16, name=f"vaug{c}")
                    for c in range(2)]
            # normalized attention output accumulator, [token, dv] layout
            oacc = [[cpool.tile([128, DV], bf16, name=f"oacc{h}_{s}") for s in range(N // 128)]
                    for h in range(H_LOC)]
            # final transposed attention output (this core's heads)
            otc = [cpool.tile([128, N], bf16, name=f"otc{i}") for i in range(2)]
            # mask, fully resident: [j-chunk partition, query] per j
            m_sb = [cpool.tile([128, N], bf16, name=f"m{j}") for j in range(NJ)]

            cc_in_h = [dpool.tile([2 * 128, N // 2], bf16, name=f"ccin{T}")
                       for T in range(2)]
            cc_out_h = [dpool.tile([4 * 128, N // 2], bf16, name=f"ccout{T}")
                        for T in range(2)]

            for c in range(2):
                nc.gpsimd.memset(vaug[c][:], 0.0)
                for j in range(NJ):
                    nc.gpsimd.memset(vaug[c][:, CW * j + 64:CW * j + 65], 1.0)
                    nc.gpsimd.memset(vaug[c][:, CW * j + 129:CW * j + 130], 1.0)

            otf_h = [[cpool.tile([128, N // 2], bf16, name=f"otf{T}_{c}")
                      for c in range(4)] for T in range(2)]

            # =================== QKV projection ===================
            with (
                tc.tile_pool(name="xs", bufs=8) as xpool,
                tc.tile_pool(name="combp", bufs=1) as combpool,
                tc.tile_pool(name="qkv_ps", bufs=3, space="PSUM") as qkv_ps,
                tc.tile_pool(name="vtr_ps", bufs=2, space="PSUM") as vtr_ps,
            ):
                # V^T combined: head h at rows 64*(h%2)+[va(32)|vp(16)|vk(16)]
                # of tile h//2; scoped to the prefix (dies after the vaug
                # transposes so its SBUF is reused by the attention pools)
                comb = [combpool.tile([128, N], bf16, name=f"comb{i}")
                        for i in range(2)]
                # u-merged [128, 1024] PSUM accumulators (2 banks each, two
                # 512-col matmuls per f-chunk); copies are per-tp2 and split
                # between Vector and Scalar (both idle in the prefix)
                # pass 1: branch a complete (q, k, v in one x stream)
                for tp2 in range(2):
                    t0 = 2 * IB * tp2
                    tsl = slice(t0, t0 + 2 * IB)
                    ps_q = qkv_ps.tile([128, 2 * IB], f32, tag="qkv", name="psq")
                    ps_k = qkv_ps.tile([128, 2 * IB], f32, tag="qkv", name="psk")
                    ps_va = qkv_ps.tile([128, 2 * IB], f32, tag="qkv", name="psva")
                    for f in range(16):
                        xt = xpool.tile([128, 2 * IB], bf16, tag="x")
                        nc.sync.dma_start(
                            xt[:], xT[128 * f:128 * (f + 1), t0:t0 + 2 * IB])
                        st, sp = (f == 0), (f == 15)
                        w = wa_sb[f]
                        for u in range(2):
                            usl = slice(IB * u, IB * (u + 1))
                            xu = xt[:, usl]
                            nc.tensor.matmul(ps_q[:, usl], w[:, 0:128], xu, start=st, stop=sp)
                            nc.tensor.matmul(ps_k[:, usl], w[:, 128:256], xu, start=st, stop=sp)
                            nc.tensor.matmul(ps_va[:, usl], w[:, 256:384], xu, start=st, stop=sp)
                    nc.vector.tensor_copy(qTa[:, tsl], ps_q[:])
                    nc.scalar.copy(kTa[:, tsl], ps_k[:])
                    for h in range(H_LOC):
                        nc.vector.tensor_copy(
                            comb[h // 2][64 * (h % 2):64 * (h % 2) + da, tsl],
                            ps_va[da * h:da * (h + 1), :])

                # mask loads overlap pass 2 (must precede the first multiply;
                # kept off the front so they don't delay pass 1's x stream)
                for j in range(NJ):
                    nc.sync.dma_start(m_sb[j][:], maskT[128 * j:128 * (j + 1), :])

                # pass 2: p+k in one x stream.  Weight tensor has 5 128-col
                # sections (qp | kp | qk | kk | v): p sections contract only
                # x rows 0:1024 (f 0..7), k sections rows 1024:2048
                # (f 8..15), v spans all 16.  qp/kp PSUM banks are copied
                # out at f==8 and their slots recycled for qk/kk.
                for tp2 in range(2):
                    t0 = 2 * IB * tp2
                    tsl = slice(t0, t0 + 2 * IB)

                    def pk_qk_copies(ps_q, ps_k, qT_d, kT_d, d_):
                        for h in range(H_LOC):
                            pb = 32 * h
                            nc.scalar.copy(qT_d[pb:pb + d_, tsl],
                                           ps_q[pb:pb + d_, :])
                            nc.scalar.copy(kT_d[pb:pb + d_, tsl],
                                           ps_k[pb:pb + d_, :])

                    # allocation order matters: the round-robin slot rotation
                    # must map the f==8 reallocation of ps_q/ps_k onto the
                    # slots freed by their own copies, not onto the still-live
                    # ps_v accumulator.
                    ps_q = qkv_ps.tile([128, 2 * IB], f32, tag="qkv", name="psq")
                    ps_k = qkv_ps.tile([128, 2 * IB], f32, tag="qkv", name="psk")
                    ps_v = qkv_ps.tile([128, 2 * IB], f32, tag="qkv", name="psv")
                    for f in range(16):
                        xt = xpool.tile([128, 2 * IB], bf16, tag="x")
                        nc.sync.dma_start(
                            xt[:], xT[DA + 128 * f:DA + 128 * (f + 1), t0:t0 + 2 * IB])
                        w = wpk_sb[f]
                        if f == 8:
                            # p-branch q/k complete: drain, recycle banks
                            pk_qk_copies(ps_q, ps_k, qTp, kTp, dp)
                            ps_q = qkv_ps.tile([128, 2 * IB], f32, tag="qkv", name="psq")
                            ps_k = qkv_ps.tile([128, 2 * IB], f32, tag="qkv", name="psk")
                        qofs = 0 if f < 8 else 256
                        st, sp = (f % 8 == 0), (f % 8 == 7)
                        for u in range(2):
                            usl = slice(IB * u, IB * (u + 1))
                            xu = xt[:, usl]
                            nc.tensor.matmul(ps_q[:, usl], w[:, qofs:qofs + 128], xu,
                                             start=st, stop=sp)
                            nc.tensor.matmul(ps_k[:, usl], w[:, qofs + 128:qofs + 256], xu,
                                             start=st, stop=sp)
                            nc.tensor.matmul(ps_v[:, usl], w[:, 512:640], xu,
                                             start=(f == 0), stop=(f == 15))
                    pk_qk_copies(ps_q, ps_k, qTk, kTk, dk)
                    for h in range(H_LOC):
                        nc.vector.tensor_copy(
                            comb[h // 2][64 * (h % 2) + da:64 * (h % 2) + 64, tsl],
                            ps_v[32 * h:32 * (h + 1), :])

                # V_aug: transpose comb chunks into the packed pair layout
                for j in range(NJ):
                    jsl = slice(128 * j, 128 * (j + 1))
                    for c in range(2):
                        tp = vtr_ps.tile([128, 128], bf16, tag="vtr")
                        nc.tensor.transpose(tp[:], comb[c][:, jsl], ident_bf[:])
                        nc.vector.tensor_copy(vaug[c][:, CW * j:CW * j + 64], tp[:, 0:64])
                        nc.vector.tensor_copy(vaug[c][:, CW * j + 65:CW * j + 129], tp[:, 64:128])

            # =================== attention ===================
            battn = [(qTa, kTa, da), (qTp, kTp, dp), (qTk, kTk, dk)]
            BLOCKS = [(bi, I) for bi in range(3) for I in range(NI)]

            with (
                tc.tile_pool(name="s_ps", bufs=2, space="PSUM") as s_pool,
                tc.tile_pool(name="o_ps", bufs=4, space="PSUM") as o_pool,
                tc.tile_pool(name="ep", bufs=5) as epool,
                tc.tile_pool(name="pp", bufs=5) as ppool,
                tc.tile_pool(name="ob", bufs=8) as opool,
                tc.tile_pool(name="rr", bufs=8) as rpool,
                tc.tile_pool(name="stg", bufs=4) as spool,
                tc.tile_pool(name="fin", bufs=2) as finpool,
            ):
                pending = []    # deferred epilogue: (bi, I, [o_sb x4])

                def flush_transposes(plan, c):
                    fbi, fI, osbs, tps = plan
                    for h in range(H_LOC):
                        nc.tensor.transpose(
                            tps[c][h][:], osbs[h][:, 128 * c:128 * (c + 1)],
                            ident_bf[0:65, 0:65])

                def flush_vchunk(plan, c):
                    fbi, fI, osbs, tps = plan
                    stg2 = {}
                    for h in range(H_LOC):
                        tp = tps[c][h]
                        r_sb = rpool.tile([128, 1], f32, tag="r")
                        nc.vector.reciprocal(r_sb[:], tp[:, 64:65])
                        at = oacc[h][4 * fI + c]
                        if fbi == 0:
                            nc.vector.tensor_scalar_mul(at[:], tp[:, 0:DV], r_sb[:])
                        elif fbi == 1:
                            nc.vector.scalar_tensor_tensor(
                                at[:], tp[:, 0:DV], r_sb[:], at[:],
                                op0=mult, op1=add)
                        else:
                            hp = h // 2
                            if hp not in stg2:
                                stg2[hp] = spool.tile([128, 128], bf16,
                                                      tag="stg", name="stg")
                            nc.vector.scalar_tensor_tensor(
                                stg2[hp][:, 64 * (h % 2):64 * (h % 2) + DV],
                                tp[:, 0:DV], r_sb[:], at[:],
                                op0=mult, op1=add)
                    if fbi == 2:
                        sl = 4 * fI + c
                        for hp in range(2):
                            nc.sync.dma_start_transpose(
                                otc[hp][:, 128 * sl:128 * (sl + 1)],
                                stg2[hp][:])

                def emit_cc(T):
                    hsl = slice(1024 * T, 1024 * (T + 1))
                    for c in range(2):
                        nc.sync.dma_start(
                            cc_in_h[T][128 * c:128 * (c + 1), :],
                            otc[c][:, hsl])
                    nc.gpsimd.collective_compute(
                        "AllGather",
                        mybir.AluOpType.bypass,
                        replica_groups=[[0, 1], [2, 3], [4, 5], [6, 7]],
                        ins=[cc_in_h[T].opt()],
                        outs=[cc_out_h[T].opt()],
                    )
                    for c in range(4):
                        nc.sync.dma_start(
                            otf_h[T][c][:],
                            cc_out_h[T][128 * c:128 * (c + 1), :])

                def outproj_chunk(T, ot, i2, pool):
                    i2sl = slice(IB * i2, IB * (i2 + 1))
                    ps = pool.tile([128, IB], f32, tag=("s" if pool is s_pool else "o"),
                                   name="fps")
                    for ic in range(4):
                        nc.tensor.matmul(
                            ps[:], wo_sb[ic][:, 128 * ot:128 * (ot + 1)],
                            otf_h[T][ic][:, i2sl],
                            start=(ic == 0), stop=(ic == 3))
                    fin = finpool.tile([128, IB], bf16, tag="fin", name="fin")
                    nc.vector.tensor_scalar_add(fin[:], ps[:],
                                                bias_sb[:, ot:ot + 1])
                    nc.sync.dma_start(
                        out[128 * ot:128 * (ot + 1),
                            1024 * T + IB * i2:1024 * T + IB * (i2 + 1)],
                        fin[:])

                for b_idx, (bi, I) in enumerate(BLOCKS):
                    qT_t, kT_t, d = battn[bi]
                    isl = slice(IB * I, IB * (I + 1))

                    # deferred epilogue of the previous block: allocate its
                    # PSUM tps tiles before this block's o_ps tiles so the
                    # round-robin slot rotation maps them onto the banks the
                    # previous block's casts free; emission is spread over
                    # j-steps 0..5 so neither PE nor Vector sees a burst.
                    plan = None
                    if pending:
                        fbi, fI, osbs = pending.pop()
                        tps = [[o_pool.tile([128, 65], bf16, tag="o",
                                            name="tps")
                                for h in range(H_LOC)] for c in range(4)]
                        plan = (fbi, fI, osbs, tps)

                    o_ps_h = [o_pool.tile([128, IB], f32, tag="o",
                                          name=f"ops{h}")
                              for h in range(H_LOC)]

                    for j in range(NJ):
                        for half in range(2):
                            s_t = s_pool.tile([128, 2 * IB], f32, tag="s",
                                              name=f"s{half}")
                            dots = []
                            for hh in range(2):
                                h = 2 * half + hh
                                pb = 32 * h
                                mm = nc.tensor.matmul(
                                    s_t[:, IB * hh:IB * (hh + 1)],
                                    kT_t[pb:pb + d, 128 * j:128 * (j + 1)],
                                    qT_t[pb:pb + d, isl],
                                    start=True, stop=True,
                                    tile_position=(pb, 0))
                                if dots:
                                    add_dep_helper(mm.ins, dots[-1].ins,
                                                   sync=False,
                                                   reason="chain dots")
                                dots.append(mm)

                            e_sb = epool.tile([128, 2 * IB], bf16, tag="e")
                            nc.scalar.activation(e_sb[:], s_t[:], Exp)

                            p_sb = ppool.tile([128, 2 * IB], bf16, tag="p")
                            m_bc = m_sb[j][:, None, isl].broadcast_to([128, 2, IB])
                            nc.vector.tensor_tensor(
                                p_sb[:].rearrange("p (g i) -> p g i", g=2),
                                e_sb[:].rearrange("p (g i) -> p g i", g=2),
                                m_bc, op=mult)

                            # PE part of the woven flush: 4 transposes every
                            # other j-step, placed where the PE would idle
                            # waiting for this half's mask multiply
                            if half == 0 and plan is not None and j in (0, 2, 4, 6):
                                flush_transposes(plan, j // 2)

                            for hh in range(2):
                                h = 2 * half + hh
                                vofs = CW * j + 65 * (h % 2)
                                nc.tensor.matmul(
                                    o_ps_h[h][:],
                                    vaug[h // 2][:, vofs:vofs + 128],
                                    p_sb[:, IB * hh:IB * (hh + 1)],
                                    start=(j == 0), stop=(j == NJ - 1),
                                    skip_group_check=True)

                            # Vector part of the woven flush (one chunk per
                            # transpose batch, a half-step later)
                            if half == 1 and plan is not None and j in (0, 2, 4, 6):
                                flush_vchunk(plan, j // 2)

                        if plan is not None and j == 9 and plan[0] == 2 and plan[1] == 1:
                            emit_cc(0)

                        # weave the T=0 output projection into the last
                        # block, one chunk per j-step (fps borrows s slots)
                        if bi == 2 and I == 3 and 6 <= j <= 13:
                            k = j - 6
                            outproj_chunk(0, k // 2, k % 2, s_pool)

                    # drain the o accumulators (V) so their banks free fast
                    osbs = []
                    for h in range(H_LOC):
                        o_sb = opool.tile([65, IB], bf16, tag="osb",
                                          name=f"osb{h}")
                        nc.vector.tensor_copy(o_sb[:], o_ps_h[h][0:65, :])
                        osbs.append(o_sb)
                    pending.append((bi, I, osbs))

                # tail: flush the last block (2,3) serially, gather the
                # second token half, and emit its output projection
                fbi, fI, osbs = pending.pop()
                tps = [[o_pool.tile([128, 65], bf16, tag="o", name="tps")
                        for h in range(H_LOC)] for c in range(4)]
                plan = (fbi, fI, osbs, tps)
                for c in range(4):
                    flush_transposes(plan, c)
                    flush_vchunk(plan, c)
                emit_cc(1)
                for ot in range(4):
                    for i2 in range(2):
                        outproj_chunk(1, ot, i2, o_pool)

    nc.compile()
    return nc


def _prep_core(c, x, W_a, W_p, W_k, W_out, b_out, mask):
    b = c // 2
    h0 = H_LOC * (c % 2)

    xT = np.ascontiguousarray(x[b].T).astype(BF16)
    maskT = np.ascontiguousarray(mask[b, 0].T).astype(BF16)

    qa = W_a[da * h0: da * (h0 + H_LOC), :] * (DA ** -0.5)
    ka = W_a[DA_H + da * h0: DA_H + da * (h0 + H_LOC), :]
    va = W_a[2 * DA_H + da * h0: 2 * DA_H + da * (h0 + H_LOC), :]
    waT = np.concatenate([qa.T, ka.T, va.T], axis=1).astype(BF16)

    # p+k branch weights, 5 128-col sections: [qp | kp | qk | kk | v].
    # Rows 0:DP come from W_p, rows DP: from W_k.  Head h of each q/k
    # section sits at cols 32h+0:16; the v section packs vp at 32h+0:16
    # and vk at 32h+16:32 (matching the comb row layout).
    wpk = np.zeros((DP + DK, 640), np.float32)
    for h in range(H_LOC):
        hh = h0 + h
        wpk[0:DP, 32 * h:32 * h + dp] = W_p[dp * hh: dp * (hh + 1), :].T * (DP ** -0.5)
        wpk[0:DP, 128 + 32 * h:128 + 32 * h + dp] = W_p[DP_H + dp * hh: DP_H + dp * (hh + 1), :].T
        wpk[DP:, 256 + 32 * h:256 + 32 * h + dk] = W_k[dk * hh: dk * (hh + 1), :].T * (DK ** -0.5)
        wpk[DP:, 384 + 32 * h:384 + 32 * h + dk] = W_k[DK_H + dk * hh: DK_H + dk * (hh + 1), :].T
        wpk[0:DP, 512 + 32 * h:512 + 32 * h + dp] = W_p[2 * DP_H + dp * hh: 2 * DP_H + dp * (hh + 1), :].T
        wpk[DP:, 512 + 32 * h + 16:512 + 32 * h + 16 + dk] = W_k[2 * DK_H + dk * hh: 2 * DK_H + dk * (hh + 1), :].T
    wpkT = wpk.astype(BF16)

    woutT = np.ascontiguousarray((W_out / 3.0).T).astype(BF16)
    bout = np.ascontiguousarray(b_out.reshape(DOUT, 1)).astype(np.float32)

    return {
        "xT": np.ascontiguousarray(xT),
        "maskT": np.ascontiguousarray(maskT),
        "waT": np.ascontiguousarray(waT),
        "wpkT": np.ascontiguousarray(wpkT),
        "woutT": woutT,
        "bout": bout,
    }


def kernel(x, W_a, W_p, W_k, W_out, b_out, mask):
    from concourse.bass_utils import run_bass_kernel_spmd

    x = np.asarray(x, np.float32)
    W_a = np.asarray(W_a, np.float32)
    W_p = np.asarray(W_p, np.float32)
    W_k = np.asarray(W_k, np.float32)
    W_out = np.asarray(W_out, np.float32)
    b_out = np.asarray(b_out, np.float32)
    mask = np.asarray(mask)

    if "nc" not in _CACHE:
        _CACHE["nc"] = _build()
    nc = _CACHE["nc"]

    in_maps = [_prep_core(c, x, W_a, W_p, W_k, W_out, b_out, mask)
               for c in range(NCORES)]
    res = run_bass_kernel_spmd(nc, in_maps, core_ids=list(range(NCORES)))

    outs = []
    for b in range(B):
        outs.append(np.asarray(res.results[2 * b]["out"]).astype(np.float32).T)
    return np.stack(outs, axis=0)


# revision 27
# speedup vs baseline: 1.0716x; 1.0716x over previous
"""Distributed Trainium2 kernel for the 3-branch masked attention problem.

Sharding: 8 cores; core c handles batch b = c//2 and heads h0 = 4*(c%2) .. +4
(data + head parallel).  Each core computes QKV for its heads, the three
branch softmaxes and AV locally, then a pair-wise AllGather of the [256, 1024]
attention output (transposed) per token half lets both cores of a batch apply
the output projection; the host reads even cores' outputs.

Pipeline design (v2): the attention inner loop is ACT(exp)-bound at
~2 us / j-step (2 exps of [128,1024]).  Everything else is organized to fit
under that: row-group-concurrent dots, one 4-head mask multiply per j-step on
DVE, and a one-block deferred epilogue (PE transposes) flushed at the top of
the next block so the o_ps PSUM rotation never stalls the exp stream.
"""

import numpy as np
import ml_dtypes

BF16 = ml_dtypes.bfloat16

H = 8
DA, DP, DK = 2048, 1024, 1024
B, N = 4, 2048
DOUT = 512
H_LOC = 4           # heads per core
DA_H, DP_H, DK_H = DA // H, DP // H, DK // H      # 256, 128, 128
da, dp, dk = DA_H // H, DP_H // H, DK_H // H      # 32, 16, 16
DV = da + dp + dk                                 # 64
NCORES = 8

IB = 512            # query block (moving dim of dots / AV)
JB = 128            # key chunk (contract chunk of AV, M of dots)
NI = N // IB        # 4
NJ = N // JB        # 16

_CACHE = {}


def _build():
    import concourse.bass as bass
    import concourse.mybir as mybir
    import concourse.tile as tile
    from concourse import bacc
    from concourse.masks import make_identity
    from concourse.tile import add_dep_helper

    f32 = mybir.dt.float32
    bf16 = mybir.dt.bfloat16
    Exp = mybir.ActivationFunctionType.Exp
    mult = mybir.AluOpType.mult
    add = mybir.AluOpType.add

    nc = bacc.Bacc("TRN2", target_bir_lowering=False, debug=False,
                   enable_asserts=False, num_devices=NCORES)

    xT = nc.dram_tensor("xT", [DA + DP + DK, N], bf16, kind="ExternalInput")
    maskT = nc.dram_tensor("maskT", [N, N], bf16, kind="ExternalInput")
    waT = nc.dram_tensor("waT", [DA, 384], bf16, kind="ExternalInput")
    wpkT = nc.dram_tensor("wpkT", [DP + DK, 640], bf16, kind="ExternalInput")
    woutT = nc.dram_tensor("woutT", [DOUT, DOUT], bf16, kind="ExternalInput")
    bout = nc.dram_tensor("bout", [DOUT, 1], f32, kind="ExternalInput")
    out = nc.dram_tensor("out", [DOUT, N], bf16, kind="ExternalOutput")

    with tile.TileContext(nc) as tc:
        with (
            tc.tile_pool(name="const", bufs=1) as cpool,
            tc.tile_pool(name="dram", bufs=1, space="DRAM") as dpool,
        ):
            # ---- constants ----
            ident_bf = cpool.tile([128, 128], bf16)
            make_identity(nc, ident_bf)

            bias_sb = cpool.tile([128, 4], f32)
            for t in range(4):
                nc.sync.dma_start(bias_sb[:, t:t + 1], bout[128 * t:128 * (t + 1), :])

            wa_sb = [cpool.tile([128, 384], bf16, name=f"wa{f}") for f in range(16)]
            for f in range(16):
                nc.sync.dma_start(wa_sb[f][:], waT[128 * f:128 * (f + 1), :])
            wpk_sb = [cpool.tile([128, 640], bf16, name=f"wpk{f}") for f in range(16)]
            for f in range(16):
                nc.sync.dma_start(wpk_sb[f][:], wpkT[128 * f:128 * (f + 1), :])
            wo_sb = [cpool.tile([128, DOUT], bf16, name=f"wo{f}") for f in range(4)]
            for f in range(4):
                nc.sync.dma_start(wo_sb[f][:], woutT[128 * f:128 * (f + 1), :])

            # ---- persistent activations ----
            # qT/kT per branch: [128, N]; heads live at 32-aligned partition
            # bases (p/k branches use rows 32h..32h+16)
            qTa = cpool.tile([128, N], bf16)
            kTa = cpool.tile([128, N], bf16)
            qTp = cpool.tile([128, N], bf16)
            kTp = cpool.tile([128, N], bf16)
            qTk = cpool.tile([128, N], bf16)
            kTk = cpool.tile([128, N], bf16)
            # V_aug packed per head PAIR: 16 chunks of 136 cols:
            # [vA(64) | onesA | vB(64) | onesB | pad(6)] (+64 tail pad).
            # Head h=2c+q reads the [128, 128] window at 136*j + 65*q:
            # out rows 0:64 = o, row 64 = denominator, rows 65:128 junk.
            CW = 136
            vaug = [cpool.tile([128, CW * NJ + 64], bf16, name=f"vaug{c}")
                    for c in range(2)]
            # normalized attention output accumulator, [token, dv] layout
            oacc = [[cpool.tile([128, DV], bf16, name=f"oacc{h}_{s}") for s in range(N // 128)]
                    for h in range(H_LOC)]
            # final transposed attention output (this core's heads)
            otc = [cpool.tile([128, N], bf16, name=f"otc{i}") for i in range(2)]
            # mask, fully resident: [j-chunk partition, query] per j
            m_sb = [cpool.tile([128, N], bf16, name=f"m{j}") for j in range(NJ)]

            cc_in_h = [dpool.tile([2 * 128, N // 2], bf16, name=f"ccin{T}")
                       for T in range(2)]
            cc_out_h = [dpool.tile([4 * 128, N // 2], bf16, name=f"ccout{T}")
                        for T in range(2)]

            for c in range(2):
                nc.gpsimd.memset(vaug[c][:], 0.0)
                for j in range(NJ):
                    nc.gpsimd.memset(vaug[c][:, CW * j + 64:CW * j + 65], 1.0)
                    nc.gpsimd.memset(vaug[c][:, CW * j + 129:CW * j + 130], 1.0)

            otf_h = [[cpool.tile([128, N // 2], bf16, name=f"otf{T}_{c}")
                      for c in range(4)] for T in range(2)]

            # =================== QKV projection ===================
            with (
                tc.tile_pool(name="xs", bufs=8) as xpool,
                tc.tile_pool(name="combp", bufs=1) as combpool,
                tc.tile_pool(name="qkv_ps", bufs=3, space="PSUM") as qkv_ps,
                tc.tile_pool(name="vtr_ps", bufs=2, space="PSUM") as vtr_ps,
            ):
                # V^T combined: head h at rows 64*(h%2)+[va(32)|vp(16)|vk(16)]
                # of tile h//2; scoped to the prefix (dies after the vaug
                # transposes so its SBUF is reused by the attention pools)
                comb = [combpool.tile([128, N], bf16, name=f"comb{i}")
                        for i in range(2)]
                # u-merged [128, 1024] PSUM accumulators (2 banks each, two
                # 512-col matmuls per f-chunk); copies are per-tp2 and split
                # between Vector and Scalar (both idle in the prefix)
                # pass 1: branch a complete (q, k, v in one x stream)
                for tp2 in range(2):
                    t0 = 2 * IB * tp2
                    tsl = slice(t0, t0 + 2 * IB)
                    ps_q = qkv_ps.tile([128, 2 * IB], f32, tag="qkv", name="psq")
                    ps_k = qkv_ps.tile([128, 2 * IB], f32, tag="qkv", name="psk")
                    ps_va = qkv_ps.tile([128, 2 * IB], f32, tag="qkv", name="psva")
                    for f in range(16):
                        xt = xpool.tile([128, 2 * IB], bf16, tag="x")
                        nc.sync.dma_start(
                            xt[:], xT[128 * f:128 * (f + 1), t0:t0 + 2 * IB])
                        st, sp = (f == 0), (f == 15)
                        w = wa_sb[f]
                        for u in range(2):
                            usl = slice(IB * u, IB * (u + 1))
                            xu = xt[:, usl]
                            nc.tensor.matmul(ps_q[:, usl], w[:, 0:128], xu, start=st, stop=sp)
                            nc.tensor.matmul(ps_k[:, usl], w[:, 128:256], xu, start=st, stop=sp)
                            nc.tensor.matmul(ps_va[:, usl], w[:, 256:384], xu, start=st, stop=sp)
                    nc.vector.tensor_copy(qTa[:, tsl], ps_q[:])
                    nc.scalar.copy(kTa[:, tsl], ps_k[:])
                    for h in range(H_LOC):
                        nc.vector.tensor_copy(
                            comb[h // 2][64 * (h % 2):64 * (h % 2) + da, tsl],
                            ps_va[da * h:da * (h + 1), :])

                # mask loads overlap pass 2 (must precede the first multiply;
                # kept off the front so they don't delay pass 1's x stream)
                for j in range(NJ):
                    nc.sync.dma_start(m_sb[j][:], maskT[128 * j:128 * (j + 1), :])

                # pass 2: p+k in one x stream.  Weight tensor has 5 128-col
                # sections (qp | kp | qk | kk | v): p sections contract only
                # x rows 0:1024 (f 0..7), k sections rows 1024:2048
                # (f 8..15), v spans all 16.  qp/kp PSUM banks are copied
                # out at f==8 and their slots recycled for qk/kk.
                for tp2 in range(2):
                    t0 = 2 * IB * tp2
                    tsl = slice(t0, t0 + 2 * IB)

                    def pk_qk_copies(ps_q, ps_k, qT_d, kT_d, d_):
                        for h in range(H_LOC):
                            pb = 32 * h
                            nc.scalar.copy(qT_d[pb:pb + d_, tsl],
                                           ps_q[pb:pb + d_, :])
                            nc.scalar.copy(kT_d[pb:pb + d_, tsl],
                                           ps_k[pb:pb + d_, :])

                    # allocation order matters: the round-robin slot rotation
                    # must map the f==8 reallocation of ps_q/ps_k onto the
                    # slots freed by their own copies, not onto the still-live
                    # ps_v accumulator.
                    ps_q = qkv_ps.tile([128, 2 * IB], f32, tag="qkv", name="psq")
                    ps_k = qkv_ps.tile([128, 2 * IB], f32, tag="qkv", name="psk")
                    ps_v = qkv_ps.tile([128, 2 * IB], f32, tag="qkv", name="psv")
                    for f in range(16):
                        xt = xpool.tile([128, 2 * IB], bf16, tag="x")
                        nc.sync.dma_start(
                            xt[:], xT[DA + 128 * f:DA + 128 * (f + 1), t0:t0 + 2 * IB])
                        w = wpk_sb[f]
                        if f == 8:
                            # p-branch q/k complete: drain, recycle banks
                            pk_qk_copies(ps_q, ps_k, qTp, kTp, dp)
                            ps_q = qkv_ps.tile([128, 2 * IB], f32, tag="qkv", name="psq")
                            ps_k = qkv_ps.tile([128, 2 * IB], f32, tag="qkv", name="psk")
                        qofs = 0 if f < 8 else 256
                        st, sp = (f % 8 == 0), (f % 8 == 7)
                        for u in range(2):
                            usl = slice(IB * u, IB * (u + 1))
                            xu = xt[:, usl]
                            nc.tensor.matmul(ps_q[:, usl], w[:, qofs:qofs + 128], xu,
                                             start=st, stop=sp)
                            nc.tensor.matmul(ps_k[:, usl], w[:, qofs + 128:qofs + 256], xu,
                                             start=st, stop=sp)
                            nc.tensor.matmul(ps_v[:, usl], w[:, 512:640], xu,
                                             start=(f == 0), stop=(f == 15))
                    pk_qk_copies(ps_q, ps_k, qTk, kTk, dk)
                    for h in range(H_LOC):
                        nc.vector.tensor_copy(
                            comb[h // 2][64 * (h % 2) + da:64 * (h % 2) + 64, tsl],
                            ps_v[32 * h:32 * (h + 1), :])

                # V_aug: transpose comb chunks into the packed pair layout
                for j in range(NJ):
                    jsl = slice(128 * j, 128 * (j + 1))
                    for c in range(2):
                        tp = vtr_ps.tile([128, 128], bf16, tag="vtr")
                        nc.tensor.transpose(tp[:], comb[c][:, jsl], ident_bf[:])
                        nc.vector.tensor_copy(vaug[c][:, CW * j:CW * j + 64], tp[:, 0:64])
                        nc.vector.tensor_copy(vaug[c][:, CW * j + 65:CW * j + 129], tp[:, 64:128])

            # =================== attention ===================
            battn = [(qTa, kTa, da), (qTp, kTp, dp), (qTk, kTk, dk)]
            BLOCKS = [(bi, I) for bi in range(3) for I in range(NI)]

            with (
                tc.tile_pool(name="s_ps", bufs=2, space="PSUM") as s_pool,
                tc.tile_pool(name="o_ps", bufs=4, space="PSUM") as o_pool,
                tc.tile_pool(name="ep", bufs=5) as epool,
                tc.tile_pool(name="pp", bufs=5) as ppool,
                tc.tile_pool(name="ob", bufs=8) as opool,
                tc.tile_pool(name="rr", bufs=8) as rpool,
                tc.tile_pool(name="stg", bufs=4) as spool,
                tc.tile_pool(name="fin", bufs=2) as finpool,
            ):
                pending = []    # deferred epilogue: (bi, I, [o_sb x4])

                def flush_transposes(plan, c, tpt):
                    fbi, fI, osbs = plan
                    for h in range(H_LOC):
                        nc.tensor.transpose(
                            tpt[:, 128 * h:128 * h + 65],
                            osbs[h][:, 128 * c:128 * (c + 1)],
                            ident_bf[0:65, 0:65])

                def flush_vchunk(plan, c, tpt):
                    fbi, fI, osbs = plan
                    stg2 = {}
                    for h in range(H_LOC):
                        tp = tpt[:, 128 * h:128 * h + 65]
                        r_sb = rpool.tile([128, 1], f32, tag="r")
                        nc.vector.reciprocal(r_sb[:], tp[:, 64:65])
                        at = oacc[h][4 * fI + c]
                        if fbi == 0:
                            nc.vector.tensor_scalar_mul(at[:], tp[:, 0:DV], r_sb[:])
                        elif fbi == 1:
                            nc.vector.scalar_tensor_tensor(
                                at[:], tp[:, 0:DV], r_sb[:], at[:],
                                op0=mult, op1=add)
                        else:
                            hp = h // 2
                            if hp not in stg2:
                                stg2[hp] = spool.tile([128, 128], bf16,
                                                      tag="stg", name="stg")
                            nc.vector.scalar_tensor_tensor(
                                stg2[hp][:, 64 * (h % 2):64 * (h % 2) + DV],
                                tp[:, 0:DV], r_sb[:], at[:],
                                op0=mult, op1=add)
                    if fbi == 2:
                        sl = 4 * fI + c
                        for hp in range(2):
                            nc.sync.dma_start_transpose(
                                otc[hp][:, 128 * sl:128 * (sl + 1)],
                                stg2[hp][:])

                def emit_cc(T):
                    hsl = slice(1024 * T, 1024 * (T + 1))
                    for c in range(2):
                        nc.sync.dma_start(
                            cc_in_h[T][128 * c:128 * (c + 1), :],
                            otc[c][:, hsl])
                    nc.gpsimd.collective_compute(
                        "AllGather",
                        mybir.AluOpType.bypass,
                        replica_groups=[[0, 1], [2, 3], [4, 5], [6, 7]],
                        ins=[cc_in_h[T].opt()],
                        outs=[cc_out_h[T].opt()],
                    )
                    for c in range(4):
                        nc.sync.dma_start(
                            otf_h[T][c][:],
                            cc_out_h[T][128 * c:128 * (c + 1), :])

                def outproj_chunk(T, ot, i2, pool):
                    i2sl = slice(IB * i2, IB * (i2 + 1))
                    ps = pool.tile([128, IB], f32, tag=("s" if pool is s_pool else "o"),
                                   name="fps")
                    for ic in range(4):
                        nc.tensor.matmul(
                            ps[:], wo_sb[ic][:, 128 * ot:128 * (ot + 1)],
                            otf_h[T][ic][:, i2sl],
                            start=(ic == 0), stop=(ic == 3))
                    fin = finpool.tile([128, IB], bf16, tag="fin", name="fin")
                    nc.vector.tensor_scalar_add(fin[:], ps[:],
                                                bias_sb[:, ot:ot + 1])
                    nc.sync.dma_start(
                        out[128 * ot:128 * (ot + 1),
                            1024 * T + IB * i2:1024 * T + IB * (i2 + 1)],
                        fin[:])

                for b_idx, (bi, I) in enumerate(BLOCKS):
                    qT_t, kT_t, d = battn[bi]
                    isl = slice(IB * I, IB * (I + 1))

                    # deferred epilogue of the previous block: its transposes
                    # write into s-pool slices (tpt) so the o_ps rotation is
                    # never delayed; one chunk per j-step over j=0..3.
                    plan = None
                    if pending:
                        plan = pending.pop()

                    o_ps_h = [o_pool.tile([128, IB], f32, tag="o",
                                          name=f"ops{h}")
                              for h in range(H_LOC)]

                    for j in range(NJ):
                        tpt = None
                        for half in range(2):
                            s_t = s_pool.tile([128, 2 * IB], f32, tag="s",
                                              name=f"s{half}")
                            dots = []
                            for hh in range(2):
                                h = 2 * half + hh
                                pb = 32 * h
                                mm = nc.tensor.matmul(
                                    s_t[:, IB * hh:IB * (hh + 1)],
                                    kT_t[pb:pb + d, 128 * j:128 * (j + 1)],
                                    qT_t[pb:pb + d, isl],
                                    start=True, stop=True,
                                    tile_position=(pb, 0))
                                if dots:
                                    add_dep_helper(mm.ins, dots[-1].ins,
                                                   sync=False,
                                                   reason="chain dots")
                                dots.append(mm)

                            e_sb = epool.tile([128, 2 * IB], bf16, tag="e")
                            nc.scalar.activation(e_sb[:], s_t[:], Exp)

                            # PE part of the woven flush, placed where the PE
                            # would idle waiting for this half's multiply
                            if half == 1 and plan is not None and j <= 3:
                                tpt = s_pool.tile([128, 512], bf16, tag="s",
                                                  name="tpt")
                                flush_transposes(plan, j, tpt)

                            p_sb = ppool.tile([128, 2 * IB], bf16, tag="p")
                            m_bc = m_sb[j][:, None, isl].broadcast_to([128, 2, IB])
                            nc.vector.tensor_tensor(
                                p_sb[:].rearrange("p (g i) -> p g i", g=2),
                                e_sb[:].rearrange("p (g i) -> p g i", g=2),
                                m_bc, op=mult)

                            for hh in range(2):
                                h = 2 * half + hh
                                vofs = CW * j + 65 * (h % 2)
                                nc.tensor.matmul(
                                    o_ps_h[h][:],
                                    vaug[h // 2][:, vofs:vofs + 128],
                                    p_sb[:, IB * hh:IB * (hh + 1)],
                                    start=(j == 0), stop=(j == NJ - 1),
                                    skip_group_check=True)

                            # Vector part of the woven flush right after this
                            # half's multiply
                            if half == 1 and plan is not None and j <= 3:
                                flush_vchunk(plan, j, tpt)

                        if plan is not None and j == 9 and plan[0] == 2 and plan[1] == 1:
                            emit_cc(0)

                        # weave the T=0 output projection into the last
                        # block, one chunk per j-step (fps borrows s slots)
                        if bi == 2 and I == 3 and 6 <= j <= 13:
                            k = j - 6
                            outproj_chunk(0, k // 2, k % 2, s_pool)

                    # drain the o accumulators (V) so their banks free fast
                    osbs = []
                    for h in range(H_LOC):
                        o_sb = opool.tile([65, IB], bf16, tag="osb",
                                          name=f"osb{h}")
                        nc.vector.tensor_copy(o_sb[:], o_ps_h[h][0:65, :])
                        osbs.append(o_sb)
                    pending.append((bi, I, osbs))

                # tail: flush the last block (2,3) serially, gather the
                # second token half, and emit its output projection
                plan = pending.pop()
                for c in range(4):
                    tpt = s_pool.tile([128, 512], bf16, tag="s", name="tpt")
                    flush_transposes(plan, c, tpt)
                    flush_vchunk(plan, c, tpt)
                emit_cc(1)
                for ot in range(4):
                    for i2 in range(2):
                        outproj_chunk(1, ot, i2, o_pool)

    nc.compile()
    return nc


def _prep_core(c, x, W_a, W_p, W_k, W_out, b_out, mask):
    b = c // 2
    h0 = H_LOC * (c % 2)

    xT = np.ascontiguousarray(x[b].T).astype(BF16)
    maskT = np.ascontiguousarray(mask[b, 0].T).astype(BF16)

    qa = W_a[da * h0: da * (h0 + H_LOC), :] * (DA ** -0.5)
    ka = W_a[DA_H + da * h0: DA_H + da * (h0 + H_LOC), :]
    va = W_a[2 * DA_H + da * h0: 2 * DA_H + da * (h0 + H_LOC), :]
    waT = np.concatenate([qa.T, ka.T, va.T], axis=1).astype(BF16)

    # p+k branch weights, 5 128-col sections: [qp | kp | qk | kk | v].
    # Rows 0:DP come from W_p, rows DP: from W_k.  Head h of each q/k
    # section sits at cols 32h+0:16; the v section packs vp at 32h+0:16
    # and vk at 32h+16:32 (matching the comb row layout).
    wpk = np.zeros((DP + DK, 640), np.float32)
    for h in range(H_LOC):
        hh = h0 + h
        wpk[0:DP, 32 * h:32 * h + dp] = W_p[dp * hh: dp * (hh + 1), :].T * (DP ** -0.5)
        wpk[0:DP, 128 + 32 * h:128 + 32 * h + dp] = W_p[DP_H + dp * hh: DP_H + dp * (hh + 1), :].T
        wpk[DP:, 256 + 32 * h:256 + 32 * h + dk] = W_k[dk * hh: dk * (hh + 1), :].T * (DK ** -0.5)
        wpk[DP:, 384 + 32 * h:384 + 32 * h + dk] = W_k[DK_H + dk * hh: DK_H + dk * (hh + 1), :].T
        wpk[0:DP, 512 + 32 * h:512 + 32 * h + dp] = W_p[2 * DP_H + dp * hh: 2 * DP_H + dp * (hh + 1), :].T
        wpk[DP:, 512 + 32 * h + 16:512 + 32 * h + 16 + dk] = W_k[2 * DK_H + dk * hh: 2 * DK_H + dk * (hh + 1), :].T
    wpkT = wpk.astype(BF16)

    woutT = np.ascontiguousarray((W_out / 3.0).T).astype(BF16)
    bout = np.ascontiguousarray(b_out.reshape(DOUT, 1)).astype(np.float32)

    return {
        "xT": np.ascontiguousarray(xT),
        "maskT": np.ascontiguousarray(maskT),
        "waT": np.ascontiguousarray(waT),
        "wpkT": np.ascontiguousarray(wpkT),
        "woutT": woutT,
        "bout": bout,
    }


def kernel(x, W_a, W_p, W_k, W_out, b_out, mask):
    from concourse.bass_utils import run_bass_kernel_spmd

    x = np.asarray(x, np.float32)
    W_a = np.asarray(W_a, np.float32)
    W_p = np.asarray(W_p, np.float32)
    W_k = np.asarray(W_k, np.float32)
    W_out = np.asarray(W_out, np.float32)
    b_out = np.asarray(b_out, np.float32)
    mask = np.asarray(mask)

    if "nc" not in _CACHE:
        _CACHE["nc"] = _build()
    nc = _CACHE["nc"]

    in_maps = [_prep_core(c, x, W_a, W_p, W_k, W_out, b_out, mask)
               for c in range(NCORES)]
    res = run_bass_kernel_spmd(nc, in_maps, core_ids=list(range(NCORES)))

    outs = []
    for b in range(B):
        outs.append(np.asarray(res.results[2 * b]["out"]).astype(np.float32).T)
    return np.stack(outs, axis=0)


# revision 29
# speedup vs baseline: 1.0879x; 1.0152x over previous
"""Distributed Trainium2 kernel for the 3-branch masked attention problem.

Sharding: 8 cores; core c handles batch b = c//2 and heads h0 = 4*(c%2) .. +4
(data + head parallel).  Each core computes QKV for its heads, the three
branch softmaxes and AV locally, then a pair-wise AllGather of the [256, 1024]
attention output (transposed) per token half lets both cores of a batch apply
the output projection; the host reads even cores' outputs.

Pipeline design (v2): the attention inner loop is ACT(exp)-bound at
~2 us / j-step (2 exps of [128,1024]).  Everything else is organized to fit
under that: row-group-concurrent dots, one 4-head mask multiply per j-step on
DVE, and a one-block deferred epilogue (PE transposes) flushed at the top of
the next block so the o_ps PSUM rotation never stalls the exp stream.
"""

import numpy as np
import ml_dtypes

BF16 = ml_dtypes.bfloat16

H = 8
DA, DP, DK = 2048, 1024, 1024
B, N = 4, 2048
DOUT = 512
H_LOC = 4           # heads per core
DA_H, DP_H, DK_H = DA // H, DP // H, DK // H      # 256, 128, 128
da, dp, dk = DA_H // H, DP_H // H, DK_H // H      # 32, 16, 16
DV = da + dp + dk                                 # 64
NCORES = 8

IB = 512            # query block (moving dim of dots / AV)
JB = 128            # key chunk (contract chunk of AV, M of dots)
NI = N // IB        # 4
NJ = N // JB        # 16

_CACHE = {}


def _build():
    import concourse.bass as bass
    import concourse.mybir as mybir
    import concourse.tile as tile
    from concourse import bacc
    from concourse.masks import make_identity
    from concourse.tile import add_dep_helper

    f32 = mybir.dt.float32
    bf16 = mybir.dt.bfloat16
    Exp = mybir.ActivationFunctionType.Exp
    mult = mybir.AluOpType.mult
    add = mybir.AluOpType.add

    nc = bacc.Bacc("TRN2", target_bir_lowering=False, debug=False,
                   enable_asserts=False, num_devices=NCORES)

    xT = nc.dram_tensor("xT", [DA + DP + DK, N], bf16, kind="ExternalInput")
    maskT = nc.dram_tensor("maskT", [N, N], bf16, kind="ExternalInput")
    waT = nc.dram_tensor("waT", [DA, 384], bf16, kind="ExternalInput")
    wpkT = nc.dram_tensor("wpkT", [DP + DK, 640], bf16, kind="ExternalInput")
    woutT = nc.dram_tensor("woutT", [DOUT, DOUT], bf16, kind="ExternalInput")
    bout = nc.dram_tensor("bout", [DOUT, 1], f32, kind="ExternalInput")
    out = nc.dram_tensor("out", [DOUT, N], bf16, kind="ExternalOutput")

    with tile.TileContext(nc) as tc:
        with (
            tc.tile_pool(name="const", bufs=1) as cpool,
            tc.tile_pool(name="dram", bufs=1, space="DRAM") as dpool,
        ):
            # ---- constants ----
            ident_bf = cpool.tile([128, 128], bf16)
            make_identity(nc, ident_bf)

            bias_sb = cpool.tile([128, 4], f32)
            for t in range(4):
                nc.sync.dma_start(bias_sb[:, t:t + 1], bout[128 * t:128 * (t + 1), :])

            # force the Exp ACT table load now, while the prefix DMAs stream,
            # so the first attention exp doesn't stall the pipeline ramp
            warm_sb = cpool.tile([128, 1], bf16)
            nc.scalar.activation(warm_sb[:], bias_sb[:, 0:1], Exp)

            wa_sb = [cpool.tile([128, 384], bf16, name=f"wa{f}") for f in range(16)]
            for f in range(16):
                nc.sync.dma_start(wa_sb[f][:], waT[128 * f:128 * (f + 1), :])
            wpk_sb = [cpool.tile([128, 640], bf16, name=f"wpk{f}") for f in range(16)]
            for f in range(16):
                nc.sync.dma_start(wpk_sb[f][:], wpkT[128 * f:128 * (f + 1), :])
            wo_sb = [cpool.tile([128, DOUT], bf16, name=f"wo{f}") for f in range(4)]
            for f in range(4):
                nc.sync.dma_start(wo_sb[f][:], woutT[128 * f:128 * (f + 1), :])

            # ---- persistent activations ----
            # qT/kT per branch: [128, N]; heads live at 32-aligned partition
            # bases (p/k branches use rows 32h..32h+16)
            qTa = cpool.tile([128, N], bf16)
            kTa = cpool.tile([128, N], bf16)
            qTp = cpool.tile([128, N], bf16)
            kTp = cpool.tile([128, N], bf16)
            qTk = cpool.tile([128, N], bf16)
            kTk = cpool.tile([128, N], bf16)
            # V_aug packed per head PAIR: 16 chunks of 136 cols:
            # [vA(64) | onesA | vB(64) | onesB | pad(6)] (+64 tail pad).
            # Head h=2c+q reads the [128, 128] window at 136*j + 65*q:
            # out rows 0:64 = o, row 64 = denominator, rows 65:128 junk.
            CW = 136
            vaug = [cpool.tile([128, CW * NJ + 64], bf16, name=f"vaug{c}")
                    for c in range(2)]
            # normalized attention output accumulator, [token, dv] layout
            oacc = [[cpool.tile([128, DV], bf16, name=f"oacc{h}_{s}") for s in range(N // 128)]
                    for h in range(H_LOC)]
            # final transposed attention output (this core's heads)
            otc = [cpool.tile([128, N], bf16, name=f"otc{i}") for i in range(2)]
            # mask, fully resident: [j-chunk partition, query] per j
            m_sb = [cpool.tile([128, N], bf16, name=f"m{j}") for j in range(NJ)]

            cc_in_h = [dpool.tile([2 * 128, N // 2], bf16, name=f"ccin{T}")
                       for T in range(2)]
            cc_out_h = [dpool.tile([4 * 128, N // 2], bf16, name=f"ccout{T}")
                        for T in range(2)]

            for c in range(2):
                nc.gpsimd.memset(vaug[c][:], 0.0)
                for j in range(NJ):
                    nc.gpsimd.memset(vaug[c][:, CW * j + 64:CW * j + 65], 1.0)
                    nc.gpsimd.memset(vaug[c][:, CW * j + 129:CW * j + 130], 1.0)

            otf_h = [[cpool.tile([128, N // 2], bf16, name=f"otf{T}_{c}")
                      for c in range(4)] for T in range(2)]

            # =================== QKV projection ===================
            with (
                tc.tile_pool(name="xs", bufs=8) as xpool,
                tc.tile_pool(name="combp", bufs=1) as combpool,
                tc.tile_pool(name="qkv_ps", bufs=3, space="PSUM") as qkv_ps,
                tc.tile_pool(name="vtr_ps", bufs=2, space="PSUM") as vtr_ps,
            ):
                # V^T combined: head h at rows 64*(h%2)+[va(32)|vp(16)|vk(16)]
                # of tile h//2; scoped to the prefix (dies after the vaug
                # transposes so its SBUF is reused by the attention pools)
                comb = [combpool.tile([128, N], bf16, name=f"comb{i}")
                        for i in range(2)]
                # u-merged [128, 1024] PSUM accumulators (2 banks each, two
                # 512-col matmuls per f-chunk); copies are per-tp2 and split
                # between Vector and Scalar (both idle in the prefix)
                # pass 1: branch a complete (q, k, v in one x stream)
                for tp2 in range(2):
                    t0 = 2 * IB * tp2
                    tsl = slice(t0, t0 + 2 * IB)
                    ps_q = qkv_ps.tile([128, 2 * IB], f32, tag="qkv", name="psq")
                    ps_k = qkv_ps.tile([128, 2 * IB], f32, tag="qkv", name="psk")
                    ps_va = qkv_ps.tile([128, 2 * IB], f32, tag="qkv", name="psva")
                    for f in range(16):
                        xt = xpool.tile([128, 2 * IB], bf16, tag="x")
                        nc.sync.dma_start(
                            xt[:], xT[128 * f:128 * (f + 1), t0:t0 + 2 * IB])
                        st, sp = (f == 0), (f == 15)
                        w = wa_sb[f]
                        for u in range(2):
                            usl = slice(IB * u, IB * (u + 1))
                            xu = xt[:, usl]
                            nc.tensor.matmul(ps_q[:, usl], w[:, 0:128], xu, start=st, stop=sp)
                            nc.tensor.matmul(ps_k[:, usl], w[:, 128:256], xu, start=st, stop=sp)
                            nc.tensor.matmul(ps_va[:, usl], w[:, 256:384], xu, start=st, stop=sp)
                    nc.vector.tensor_copy(qTa[:, tsl], ps_q[:])
                    nc.scalar.copy(kTa[:, tsl], ps_k[:])
                    for h in range(H_LOC):
                        nc.vector.tensor_copy(
                            comb[h // 2][64 * (h % 2):64 * (h % 2) + da, tsl],
                            ps_va[da * h:da * (h + 1), :])

                # mask loads overlap pass 2 (must precede the first multiply;
                # kept off the front so they don't delay pass 1's x stream)
                for j in range(NJ):
                    nc.sync.dma_start(m_sb[j][:], maskT[128 * j:128 * (j + 1), :])

                # pass 2: p+k in one x stream.  Weight tensor has 5 128-col
                # sections (qp | kp | qk | kk | v): p sections contract only
                # x rows 0:1024 (f 0..7), k sections rows 1024:2048
                # (f 8..15), v spans all 16.  qp/kp PSUM banks are copied
                # out at f==8 and their slots recycled for qk/kk.
                for tp2 in range(2):
                    t0 = 2 * IB * tp2
                    tsl = slice(t0, t0 + 2 * IB)

                    def pk_qk_copies(ps_q, ps_k, qT_d, kT_d, d_):
                        for h in range(H_LOC):
                            pb = 32 * h
                            nc.scalar.copy(qT_d[pb:pb + d_, tsl],
                                           ps_q[pb:pb + d_, :])
                            nc.scalar.copy(kT_d[pb:pb + d_, tsl],
                                           ps_k[pb:pb + d_, :])

                    # allocation order matters: the round-robin slot rotation
                    # must map the f==8 reallocation of ps_q/ps_k onto the
                    # slots freed by their own copies, not onto the still-live
                    # ps_v accumulator.
                    ps_q = qkv_ps.tile([128, 2 * IB], f32, tag="qkv", name="psq")
                    ps_k = qkv_ps.tile([128, 2 * IB], f32, tag="qkv", name="psk")
                    ps_v = qkv_ps.tile([128, 2 * IB], f32, tag="qkv", name="psv")
                    for f in range(16):
                        xt = xpool.tile([128, 2 * IB], bf16, tag="x")
                        nc.sync.dma_start(
                            xt[:], xT[DA + 128 * f:DA + 128 * (f + 1), t0:t0 + 2 * IB])
                        w = wpk_sb[f]
                        if f == 8:
                            # p-branch q/k complete: drain, recycle banks
                            pk_qk_copies(ps_q, ps_k, qTp, kTp, dp)
                            ps_q = qkv_ps.tile([128, 2 * IB], f32, tag="qkv", name="psq")
                            ps_k = qkv_ps.tile([128, 2 * IB], f32, tag="qkv", name="psk")
                        qofs = 0 if f < 8 else 256
                        st, sp = (f % 8 == 0), (f % 8 == 7)
                        for u in range(2):
                            usl = slice(IB * u, IB * (u + 1))
                            xu = xt[:, usl]
                            nc.tensor.matmul(ps_q[:, usl], w[:, qofs:qofs + 128], xu,
                                             start=st, stop=sp)
                            nc.tensor.matmul(ps_k[:, usl], w[:, qofs + 128:qofs + 256], xu,
                                             start=st, stop=sp)
                            nc.tensor.matmul(ps_v[:, usl], w[:, 512:640], xu,
                                             start=(f == 0), stop=(f == 15))
                    pk_qk_copies(ps_q, ps_k, qTk, kTk, dk)
                    for h in range(H_LOC):
                        nc.vector.tensor_copy(
                            comb[h // 2][64 * (h % 2) + da:64 * (h % 2) + 64, tsl],
                            ps_v[32 * h:32 * (h + 1), :])

                    # V_aug for this token half: transpose comb chunks into
                    # the packed pair layout.  Interleaved per tp2 so the PE
                    # never idles at the QKV -> attention transition (an idle
                    # here demotes the PE clock for the whole attention span).
                    for j in range(8 * tp2, 8 * tp2 + 8):
                        jsl = slice(128 * j, 128 * (j + 1))
                        for c in range(2):
                            tp = vtr_ps.tile([128, 128], bf16, tag="vtr")
                            nc.tensor.transpose(tp[:], comb[c][:, jsl], ident_bf[:])
                            nc.vector.tensor_copy(vaug[c][:, CW * j:CW * j + 64], tp[:, 0:64])
                            nc.vector.tensor_copy(vaug[c][:, CW * j + 65:CW * j + 129], tp[:, 64:128])

            # =================== attention ===================
            battn = [(qTa, kTa, da), (qTp, kTp, dp), (qTk, kTk, dk)]
            BLOCKS = [(bi, I) for bi in range(3) for I in range(NI)]

            with (
                tc.tile_pool(name="s_ps", bufs=2, space="PSUM") as s_pool,
                tc.tile_pool(name="o_ps", bufs=4, space="PSUM") as o_pool,
                tc.tile_pool(name="ep", bufs=5) as epool,
                tc.tile_pool(name="pp", bufs=5) as ppool,
                tc.tile_pool(name="ob", bufs=8) as opool,
                tc.tile_pool(name="rr", bufs=8) as rpool,
                tc.tile_pool(name="stg", bufs=4) as spool,
                tc.tile_pool(name="fin", bufs=2) as finpool,
            ):
                pending = []    # deferred epilogue: (bi, I, [o_sb x4])

                def flush_transposes(plan, c, tpt):
                    fbi, fI, osbs = plan
                    for h in range(H_LOC):
                        nc.tensor.transpose(
                            tpt[:, 128 * h:128 * h + 65],
                            osbs[h][:, 128 * c:128 * (c + 1)],
                            ident_bf[0:65, 0:65])

                def flush_vchunk(plan, c, tpt):
                    fbi, fI, osbs = plan
                    stg2 = {}
                    for h in range(H_LOC):
                        tp = tpt[:, 128 * h:128 * h + 65]
                        r_sb = rpool.tile([128, 1], f32, tag="r")
                        nc.vector.reciprocal(r_sb[:], tp[:, 64:65])
                        at = oacc[h][4 * fI + c]
                        if fbi == 0:
                            nc.vector.tensor_scalar_mul(at[:], tp[:, 0:DV], r_sb[:])
                        elif fbi == 1:
                            nc.vector.scalar_tensor_tensor(
                                at[:], tp[:, 0:DV], r_sb[:], at[:],
                                op0=mult, op1=add)
                        else:
                            hp = h // 2
                            if hp not in stg2:
                                stg2[hp] = spool.tile([128, 128], bf16,
                                                      tag="stg", name="stg")
                            nc.vector.scalar_tensor_tensor(
                                stg2[hp][:, 64 * (h % 2):64 * (h % 2) + DV],
                                tp[:, 0:DV], r_sb[:], at[:],
                                op0=mult, op1=add)
                    if fbi == 2:
                        sl = 4 * fI + c
                        for hp in range(2):
                            nc.sync.dma_start_transpose(
                                otc[hp][:, 128 * sl:128 * (sl + 1)],
                                stg2[hp][:])

                def emit_cc(T):
                    hsl = slice(1024 * T, 1024 * (T + 1))
                    for c in range(2):
                        nc.sync.dma_start(
                            cc_in_h[T][128 * c:128 * (c + 1), :],
                            otc[c][:, hsl])
                    nc.gpsimd.collective_compute(
                        "AllGather",
                        mybir.AluOpType.bypass,
                        replica_groups=[[0, 1], [2, 3], [4, 5], [6, 7]],
                        ins=[cc_in_h[T].opt()],
                        outs=[cc_out_h[T].opt()],
                    )
                    for c in range(4):
                        nc.sync.dma_start(
                            otf_h[T][c][:],
                            cc_out_h[T][128 * c:128 * (c + 1), :])

                def outproj_chunk(T, ot, i2, pool):
                    i2sl = slice(IB * i2, IB * (i2 + 1))
                    ps = pool.tile([128, IB], f32, tag=("s" if pool is s_pool else "o"),
                                   name="fps")
                    for ic in range(4):
                        nc.tensor.matmul(
                            ps[:], wo_sb[ic][:, 128 * ot:128 * (ot + 1)],
                            otf_h[T][ic][:, i2sl],
                            start=(ic == 0), stop=(ic == 3))
                    fin = finpool.tile([128, IB], bf16, tag="fin", name="fin")
                    nc.vector.tensor_scalar_add(fin[:], ps[:],
                                                bias_sb[:, ot:ot + 1])
                    nc.sync.dma_start(
                        out[128 * ot:128 * (ot + 1),
                            1024 * T + IB * i2:1024 * T + IB * (i2 + 1)],
                        fin[:])

                for b_idx, (bi, I) in enumerate(BLOCKS):
                    qT_t, kT_t, d = battn[bi]
                    isl = slice(IB * I, IB * (I + 1))

                    # deferred epilogue of the previous block: its transposes
                    # write into s-pool slices (tpt) so the o_ps rotation is
                    # never delayed; one chunk per j-step over j=0..3.
                    plan = None
                    if pending:
                        plan = pending.pop()

                    o_ps_h = [o_pool.tile([128, IB], f32, tag="o",
                                          name=f"ops{h}")
                              for h in range(H_LOC)]

                    for j in range(NJ):
                        tpt = None
                        for half in range(2):
                            s_t = s_pool.tile([128, 2 * IB], f32, tag="s",
                                              name=f"s{half}")
                            dots = []
                            for hh in range(2):
                                h = 2 * half + hh
                                pb = 32 * h
                                mm = nc.tensor.matmul(
                                    s_t[:, IB * hh:IB * (hh + 1)],
                                    kT_t[pb:pb + d, 128 * j:128 * (j + 1)],
                                    qT_t[pb:pb + d, isl],
                                    start=True, stop=True,
                                    tile_position=(pb, 0))
                                if dots:
                                    add_dep_helper(mm.ins, dots[-1].ins,
                                                   sync=False,
                                                   reason="chain dots")
                                dots.append(mm)

                            e_sb = epool.tile([128, 2 * IB], bf16, tag="e")
                            nc.scalar.activation(e_sb[:], s_t[:], Exp)

                            # PE part of the woven flush, placed where the PE
                            # would idle waiting for this half's multiply
                            if half == 1 and plan is not None and j <= 3:
                                tpt = s_pool.tile([128, 512], bf16, tag="s",
                                                  name="tpt")
                                flush_transposes(plan, j, tpt)

                            p_sb = ppool.tile([128, 2 * IB], bf16, tag="p")
                            m_bc = m_sb[j][:, None, isl].broadcast_to([128, 2, IB])
                            nc.vector.tensor_tensor(
                                p_sb[:].rearrange("p (g i) -> p g i", g=2),
                                e_sb[:].rearrange("p (g i) -> p g i", g=2),
                                m_bc, op=mult)

                            for hh in range(2):
                                h = 2 * half + hh
                                vofs = CW * j + 65 * (h % 2)
                                nc.tensor.matmul(
                                    o_ps_h[h][:],
                                    vaug[h // 2][:, vofs:vofs + 128],
                                    p_sb[:, IB * hh:IB * (hh + 1)],
                                    start=(j == 0), stop=(j == NJ - 1),
                                    skip_group_check=True)

                            # Vector part of the woven flush right after this
                            # half's multiply
                            if half == 1 and plan is not None and j <= 3:
                                flush_vchunk(plan, j, tpt)

                        if plan is not None and j == 9 and plan[0] == 2 and plan[1] == 1:
                            emit_cc(0)

                        # weave the T=0 output projection into the last
                        # block, one chunk per j-step (fps borrows s slots)
                        if bi == 2 and I == 3 and 6 <= j <= 13:
                            k = j - 6
                            outproj_chunk(0, k // 2, k % 2, s_pool)

                    # drain the o accumulators (V) so their banks free fast
                    osbs = []
                    for h in range(H_LOC):
                        o_sb = opool.tile([65, IB], bf16, tag="osb",
                                          name=f"osb{h}")
                        nc.vector.tensor_copy(o_sb[:], o_ps_h[h][0:65, :])
                        osbs.append(o_sb)
                    pending.append((bi, I, osbs))

                # tail: flush the last block (2,3) serially, gather the
                # second token half, and emit its output projection
                plan = pending.pop()
                for c in range(4):
                    tpt = s_pool.tile([128, 512], bf16, tag="s", name="tpt")
                    flush_transposes(plan, c, tpt)
                    flush_vchunk(plan, c, tpt)
                emit_cc(1)
                for ot in range(4):
                    for i2 in range(2):
                        outproj_chunk(1, ot, i2, o_pool)

    nc.compile()
    return nc


def _prep_core(c, x, W_a, W_p, W_k, W_out, b_out, mask):
    b = c // 2
    h0 = H_LOC * (c % 2)

    xT = np.ascontiguousarray(x[b].T).astype(BF16)
    maskT = np.ascontiguousarray(mask[b, 0].T).astype(BF16)

    qa = W_a[da * h0: da * (h0 + H_LOC), :] * (DA ** -0.5)
    ka = W_a[DA_H + da * h0: DA_H + da * (h0 + H_LOC), :]
    va = W_a[2 * DA_H + da * h0: 2 * DA_H + da * (h0 + H_LOC), :]
    waT = np.concatenate([qa.T, ka.T, va.T], axis=1).astype(BF16)

    # p+k branch weights, 5 128-col sections: [qp | kp | qk | kk | v].
    # Rows 0:DP come from W_p, rows DP: from W_k.  Head h of each q/k
    # section sits at cols 32h+0:16; the v section packs vp at 32h+0:16
    # and vk at 32h+16:32 (matching the comb row layout).
    wpk = np.zeros((DP + DK, 640), np.float32)
    for h in range(H_LOC):
        hh = h0 + h
        wpk[0:DP, 32 * h:32 * h + dp] = W_p[dp * hh: dp * (hh + 1), :].T * (DP ** -0.5)
        wpk[0:DP, 128 + 32 * h:128 + 32 * h + dp] = W_p[DP_H + dp * hh: DP_H + dp * (hh + 1), :].T
        wpk[DP:, 256 + 32 * h:256 + 32 * h + dk] = W_k[dk * hh: dk * (hh + 1), :].T * (DK ** -0.5)
        wpk[DP:, 384 + 32 * h:384 + 32 * h + dk] = W_k[DK_H + dk * hh: DK_H + dk * (hh + 1), :].T
        wpk[0:DP, 512 + 32 * h:512 + 32 * h + dp] = W_p[2 * DP_H + dp * hh: 2 * DP_H + dp * (hh + 1), :].T
        wpk[DP:, 512 + 32 * h + 16:512 + 32 * h + 16 + dk] = W_k[2 * DK_H + dk * hh: 2 * DK_H + dk * (hh + 1), :].T
    wpkT = wpk.astype(BF16)

    woutT = np.ascontiguousarray((W_out / 3.0).T).astype(BF16)
    bout = np.ascontiguousarray(b_out.reshape(DOUT, 1)).astype(np.float32)

    return {
        "xT": np.ascontiguousarray(xT),
        "maskT": np.ascontiguousarray(maskT),
        "waT": np.ascontiguousarray(waT),
        "wpkT": np.ascontiguousarray(wpkT),
        "woutT": woutT,
        "bout": bout,
    }


def kernel(x, W_a, W_p, W_k, W_out, b_out, mask):
    from concourse.bass_utils import run_bass_kernel_spmd

    x = np.asarray(x, np.float32)
    W_a = np.asarray(W_a, np.float32)
    W_p = np.asarray(W_p, np.float32)
    W_k = np.asarray(W_k, np.float32)
    W_out = np.asarray(W_out, np.float32)
    b_out = np.asarray(b_out, np.float32)
    mask = np.asarray(mask)

    if "nc" not in _CACHE:
        _CACHE["nc"] = _build()
    nc = _CACHE["nc"]

    in_maps = [_prep_core(c, x, W_a, W_p, W_k, W_out, b_out, mask)
               for c in range(NCORES)]
    res = run_bass_kernel_spmd(nc, in_maps, core_ids=list(range(NCORES)))

    outs = []
    for b in range(B):
        outs.append(np.asarray(res.results[2 * b]["out"]).astype(np.float32).T)
    return np.stack(outs, axis=0)
